# revision 29
# baseline (speedup 1.0000x reference)
"""Trainium2 Bass kernel for nn_BilinearInterpolation_60670708023631.

Math: the reference pads the (128,128,32) image into a (128,128,65,32) volume
that is zero everywhere except depth slab z=32, trilinearly samples it at
64*64*65 transformed grid points, and sums over the 65 depth samples per
output pixel.  Because the volume is a single slab, each sample reduces to a
2D 4-corner gather weighted by a z-slab weight zw = fz0*[z0==32]+fz1*[z1==32].
The 4 corners always live in the 2x2 patch at (y0, x0), so we gather one
512-byte patch-table row per sample and fold corner selection into 4 weights.

zw is nonzero only where the (affine in k) z coordinate crosses [31, 33) —
for a given transformation that is a contiguous window of at most
floor(2/|dz/dk|)+1 of the 65 depth samples per pixel.  The kernel computes
the per-pixel window start on device and gathers/reduces only kw samples per
pixel; kw is chosen host-side from the transformation's z-slope (falling back
to wider windows or the fully dense variant when the slope is shallow), so
the result is exact for every input.

The grid coordinates replicate the reference's fp path bit-exactly
(XLA computes the 3x4 einsum as a sequential fma chain): the pixel part
fma(T[r,1], yg, fl(T[r,0]*xg)) is host side-data, and the k-dependent
fma(T[r,2], zlin, A2) is reproduced on device with exact split products
(zlin has <=6-bit mantissas) plus a 2Sum-compensated add.  This matters
because the module's x/y-swapped corner weighting is discontinuous at integer
coordinates, so a 1-ulp coordinate difference can move the output by O(1).

Sharding: 4096 output pixels split across 8 cores (512 each); the patch table
is replicated.  Per core: DVE computes coordinates/indices, the indices are
rewrapped (via a DRAM bounce) into dma_gather's 16-partition wrapped layout,
and four chunked gathers overlap the Q7 descriptor generation with the DVE
weight multiply and per-slot reduction.
"""
import numpy as np

import concourse.bass as bass
import concourse.bacc as bacc
import concourse.mybir as mybir
import concourse.tile as tile
from concourse import bass_utils, library_config

P = 128          # partitions
KD = 65          # depth samples per pixel
NS = 4           # pixel slots per partition (512 pixels / 128)
C = 32           # channels
N_CORES = 8
OUT_H = OUT_W = 64
H = W = 128

f32 = mybir.dt.float32
i32 = mybir.dt.int32
i16 = mybir.dt.int16
OP = mybir.AluOpType
AF = mybir.ActivationFunctionType

_CACHE: dict = {}

# jnp.linspace(-1, 1, 64, dtype=float32), bit-exact (differs from np.linspace)
_XY_LIN_HEX = (
    "000080bf7edf77bffcbe6fbf7a9e67bff87d5fbf765d57bff43c4fbf721c47bf"
    "f0fb3ebf6edb36bfecba2ebf6a9a26bfe8791ebf655916bfe4380ebf611806bf"
    "bfeffbbeb9aeebbeb76ddbbeb12ccbbeafebbabea9aaaabea7699abea1288abe"
    "39cf73be314d53be29cb32be214912be318ee3bd218aa2bd210c43bd010882bc"
    "4008823c400c433d308aa23d418ee33d2849123e31cb323e394d533e41cf733e"
    "a4288a3ea9699a3eadaaaa3eb1ebba3eb52ccb3eb96ddb3ebdaeeb3ec1effb3e"
    "6418063fe6380e3f6859163fea791e3f6c9a263feeba2e3f70db363ff2fb3e3f"
    "741c473ff63c4f3f785d573ffa7d5f3f7c9e673ffebe6f3f80df773f0000803f"
)
XY_LIN = np.frombuffer(bytes.fromhex(_XY_LIN_HEX), dtype=np.float32)

NH = 1           # pipeline halves (slots per half = NS // NH)


def _fma32(a, b, c):
    """float32 fused multiply-add via exact float64 intermediate."""
    return np.float32(np.float64(a) * np.float64(b) + np.float64(c))


def _split12(a):
    """Dekker split of float32 into high/low halves (<=12 mantissa bits each)."""
    a = np.float32(a)
    c = np.float32(a * np.float32(2 ** 12 + 1))
    ah = np.float32(c - np.float32(c - a))
    return ah, np.float32(a - ah)


def _build_program(kw, debug_taps=False):
    """kw = depth-window size per pixel; kw == KD means dense (no windowing)."""
    dense = kw == KD
    F = NS * kw                  # gathered points per partition
    SH = NS // NH                # slots per half
    FH = SH * kw                 # f-columns per half
    nc = bacc.Bacc("TRN2", target_bir_lowering=False, debug=False)

    tab = nc.dram_tensor("tab", (H * W, 4 * C), f32, kind="ExternalInput")
    trep = nc.dram_tensor("trep", (P, 14), f32, kind="ExternalInput")
    base2 = nc.dram_tensor("base2", (P, 3 * NS), f32, kind="ExternalInput")
    jr = nc.dram_tensor("jr", (P, FH), f32, kind="ExternalInput")  # j/z ramp
    scrs = [nc.dram_tensor(f"scr{h}", (P, FH), i16) for h in range(NH)]
    out_d = nc.dram_tensor("out", (NS, P, C), f32, kind="ExternalOutput")
    if debug_taps:
        dbg_idx = nc.dram_tensor("dbg_idx", (P, F), i16, kind="ExternalOutput")
        dbg_w = nc.dram_tensor("dbg_w", (P, F * 4), f32, kind="ExternalOutput")
        dbg_kst = nc.dram_tensor("dbg_kst", (P, NS), f32, kind="ExternalOutput")
        dbg_z = nc.dram_tensor("dbg_z", (P, F), f32, kind="ExternalOutput")

    with tile.TileContext(nc) as tc:
        with (
            tc.tile_pool(name="const", bufs=1) as cp,
            tc.tile_pool(name="work", bufs=1) as wp,
            tc.tile_pool(name="gath", bufs=3) as gp,
            tc.tile_pool(name="tmp", bufs=2) as tp,
            tc.tile_pool(name="outp", bufs=2) as op_,
        ):
            nc.gpsimd.load_library(library_config.mlp)

            # ---- load constants
            t_t = cp.tile([P, 14], f32)
            nc.sync.dma_start(out=t_t[:], in_=trep[:])
            b2_t = cp.tile([P, 3 * NS], f32)
            nc.scalar.dma_start(out=b2_t[:], in_=base2[:])
            jr_t = cp.tile([P, FH], f32)
            nc.scalar.dma_start(out=jr_t[:], in_=jr[:])

            def tcol(j):
                return t_t[:, j:j + 1]

            # ceil(v) for any v: r = rne_int(v); ceil = r + (r < v)
            def ceil_(x, tg, shape):
                ri = wp.tile(shape, i32, tag=f"cl_ri{tg}")
                nc.vector.tensor_copy(out=ri[:], in_=x[:])
                r = wp.tile(shape, f32, tag=f"cl_r{tg}")
                nc.vector.tensor_copy(out=r[:], in_=ri[:])
                g_ = wp.tile(shape, f32, tag=f"cl_g{tg}")
                nc.vector.tensor_tensor(out=g_[:], in0=r[:], in1=x[:], op=OP.is_lt)
                nc.vector.tensor_tensor(out=r[:], in0=r[:], in1=g_[:], op=OP.add)
                return r

            # floor(|x|) for trunc: r = rne_int(a); floor = r - (r > a)
            def trunc_(x, tg, shape):
                a_ = wp.tile(shape, f32, tag=f"tr_a{tg}")
                nc.scalar.activation(out=a_[:], in_=x[:], func=AF.Abs)
                ri = wp.tile(shape, i32, tag=f"tr_ri{tg}")
                nc.vector.tensor_copy(out=ri[:], in_=a_[:])
                r = wp.tile(shape, f32, tag=f"tr_r{tg}")
                nc.vector.tensor_copy(out=r[:], in_=ri[:])
                g_ = wp.tile(shape, f32, tag=f"tr_g{tg}")
                nc.vector.tensor_tensor(out=g_[:], in0=r[:], in1=a_[:], op=OP.is_gt)
                nc.vector.tensor_tensor(out=r[:], in0=r[:], in1=g_[:],
                                        op=OP.subtract)
                sg = wp.tile(shape, f32, tag=f"tr_s{tg}")
                nc.scalar.activation(out=sg[:], in_=x[:], func=AF.Sign)
                xt = wp.tile(shape, f32, tag=f"tr_t{tg}")
                nc.vector.tensor_tensor(out=xt[:], in0=r[:], in1=sg[:], op=OP.mult)
                return xt

            # trep cols: [t2h,t2l,t3]*3 coords, then t2_z full (9), rcz (10)
            # base2 cols: A2[coord r, slot t] = fma(T[r,1], yg, fl(T[r,0]*xg))
            def a2col(r, s0):
                return b2_t[:, r * NS + s0:r * NS + s0 + SH]

            # ---------- per-half phase 1: window + coords + idx + rewrap ----
            halves = []

            def phase1(h):
                s0 = h * SH
                if dense:
                    kst = None
                else:
                    # window start per slot (approximate math is fine: it only
                    # positions the window; extra samples have zw == 0 exactly)
                    z0 = wp.tile([P, SH], f32, tag=f"z0{h}")
                    nc.vector.tensor_scalar(out=z0[:], in0=a2col(2, s0),
                                            scalar1=tcol(9), scalar2=tcol(8),
                                            op0=OP.subtract, op1=OP.add)
                    nc.vector.tensor_scalar(out=z0[:], in0=z0[:], scalar1=1.0,
                                            scalar2=32.5, op0=OP.add, op1=OP.mult)
                    a = wp.tile([P, SH], f32, tag=f"wa{h}")
                    nc.vector.tensor_scalar(out=a[:], in0=z0[:], scalar1=-1.0,
                                            scalar2=31.0, op0=OP.mult, op1=OP.add)
                    nc.vector.tensor_scalar(out=a[:], in0=a[:], scalar1=tcol(10),
                                            scalar2=None, op0=OP.mult)
                    b = wp.tile([P, SH], f32, tag=f"wb{h}")
                    nc.vector.tensor_scalar(out=b[:], in0=z0[:], scalar1=-1.0,
                                            scalar2=33.0, op0=OP.mult, op1=OP.add)
                    nc.vector.tensor_scalar(out=b[:], in0=b[:], scalar1=tcol(10),
                                            scalar2=None, op0=OP.mult)
                    nc.vector.tensor_tensor(out=a[:], in0=a[:], in1=b[:], op=OP.min)
                    kc = ceil_(a, f"k{h}", [P, SH])
                    kst = wp.tile([P, SH], f32, tag=f"kst{h}")
                    nc.vector.tensor_scalar(out=kst[:], in0=kc[:], scalar1=0.0,
                                            scalar2=float(KD - kw), op0=OP.max,
                                            op1=OP.min)
                    if debug_taps:
                        nc.sync.dma_start(out=dbg_kst[:, s0:s0 + SH], in_=kst[:])

                # coordinates: bit-exact replication of the reference fp path
                #   s = fma(t3, 1, fma(t2, zl, A2)); coord = scale2 * fl(s + 1)
                if dense:
                    zl = jr_t
                else:
                    u_ = wp.tile([P, FH], f32, tag=f"u{h}")
                    nc.vector.tensor_tensor(
                        out=u_[:].rearrange("p (t k) -> p t k", t=SH),
                        in0=jr_t[:].rearrange("p (t k) -> p t k", t=SH),
                        in1=kst[:].broadcast_to([P, SH, kw]),
                        op=OP.add)
                    zl = wp.tile([P, FH], f32, tag=f"zl{h}")
                    nc.vector.tensor_scalar(out=zl[:], in0=u_[:],
                                            scalar1=1.0 / 32.0, scalar2=-1.0,
                                            op0=OP.mult, op1=OP.add)
                # batched over all 3 coords as [P, 3*FH]:
                # in0 zl broadcast over the coord dim; per-coord scalars come
                # from trep column-triples broadcast over the (slot, k) dims
                def zlb():      # (p, coord, f)
                    return bass.AP(zl[:].tensor, zl[:].offset,
                                   [zl[:].ap[0], [0, 3], [1, FH]])

                def colb(c0):   # trep cols [c0, c0+3) broadcast over f
                    v = t_t[:, c0:c0 + 3]
                    return bass.AP(v.tensor, v.offset,
                                   [v.ap[0], [1, 3], [0, FH]])

                def a2b3():     # (p, coord, slot, k) from b2 cols
                    v = b2_t[:, s0:s0 + SH]
                    return bass.AP(v.tensor, v.offset,
                                   [v.ap[0], [NS, 3], [1, SH], [0, kw]])

                def v4(t_):     # [P, 3*FH] view as (p, coord, slot, k)
                    return t_[:].rearrange("p (c t k) -> p c t k", c=3, t=SH)

                def v3c(t_):    # [P, 3*FH] view as (p, coord, f)
                    return t_[:].rearrange("p (c f) -> p c f", c=3)

                Ph = wp.tile([P, 3 * FH], f32, tag=f"cPh{h}")
                nc.vector.tensor_tensor(out=v3c(Ph), in0=zlb(), in1=colb(0),
                                        op=OP.mult)
                Pl = wp.tile([P, 3 * FH], f32, tag=f"cPl{h}")
                nc.vector.tensor_tensor(out=v3c(Pl), in0=zlb(), in1=colb(3),
                                        op=OP.mult)
                # Knuth 2Sum(A2, Ph) -> u, er;  A3 = fl(u + fl(er + Pl))
                u = wp.tile([P, 3 * FH], f32, tag=f"cu{h}")
                nc.vector.tensor_tensor(out=v4(u), in0=v4(Ph), in1=a2b3(),
                                        op=OP.add)
                bv = wp.tile([P, 3 * FH], f32, tag=f"cbv{h}")
                nc.vector.tensor_tensor(out=v4(bv), in0=v4(u), in1=a2b3(),
                                        op=OP.subtract)
                av = wp.tile([P, 3 * FH], f32, tag=f"cav{h}")
                nc.vector.tensor_tensor(out=av[:], in0=u[:], in1=bv[:],
                                        op=OP.subtract)
                br = wp.tile([P, 3 * FH], f32, tag=f"cbr{h}")
                nc.vector.tensor_tensor(out=br[:], in0=Ph[:], in1=bv[:],
                                        op=OP.subtract)
                ar = wp.tile([P, 3 * FH], f32, tag=f"car{h}")
                nc.vector.tensor_tensor(out=v4(ar), in0=v4(av), in1=a2b3(),
                                        op=OP.subtract)  # = -(A2 - av)
                er = wp.tile([P, 3 * FH], f32, tag=f"cer{h}")
                nc.vector.tensor_tensor(out=er[:], in0=br[:], in1=ar[:],
                                        op=OP.subtract)
                nc.vector.tensor_tensor(out=er[:], in0=er[:], in1=Pl[:],
                                        op=OP.add)
                nc.vector.tensor_tensor(out=u[:], in0=u[:], in1=er[:],
                                        op=OP.add)
                # v = fl(fl(A3 + t3) + 1); coord = scale2 * v
                nc.vector.tensor_tensor(out=v3c(u), in0=v3c(u), in1=colb(6),
                                        op=OP.add)
                nc.vector.tensor_scalar(out=u[:], in0=u[:], scalar1=1.0,
                                        scalar2=None, op0=OP.add)
                CO = wp.tile([P, 3 * FH], f32, tag=f"CO{h}")
                nc.vector.tensor_tensor(out=v3c(CO), in0=v3c(u), in1=colb(11),
                                        op=OP.mult)
                if debug_taps:
                    nc.sync.dma_start(out=dbg_z[:, h * FH:(h + 1) * FH],
                                      in_=CO[:, 2 * FH:3 * FH])

                # trunc + clip0 -> gather indices
                T3 = trunc_(CO, f"{h}", [P, 3 * FH])
                CF0 = wp.tile([P, 3 * FH], f32, tag=f"CF0{h}")
                nc.vector.tensor_scalar(out=CF0[:], in0=T3[:], scalar1=0.0,
                                        scalar2=127.0, op0=OP.max, op1=OP.min)
                nc.vector.tensor_scalar(out=CF0[:, 2 * FH:3 * FH],
                                        in0=CF0[:, 2 * FH:3 * FH], scalar1=64.0,
                                        scalar2=None, op0=OP.min)
                idxf = wp.tile([P, FH], f32, tag=f"idxf{h}")
                nc.vector.tensor_scalar(out=idxf[:], in0=CF0[:, FH:2 * FH],
                                        scalar1=128.0, scalar2=None, op0=OP.mult)
                idxi = wp.tile([P, FH], i16, tag=f"idxi{h}")
                nc.vector.tensor_tensor(out=idxi[:], in0=idxf[:],
                                        in1=CF0[:, 0:FH], op=OP.add)
                if debug_taps:
                    nc.sync.dma_start(out=dbg_idx[:, h * FH:(h + 1) * FH],
                                      in_=idxi[:])

                # rewrap into dma_gather's wrapped layout:
                # wrapped[q + 16r, f*8 + w] = idxi[16w + q, f]
                nc.sync.dma_start(out=scrs[h][:], in_=idxi[:])
                wT = wp.tile([P, FH * 8], i16, tag=f"wT{h}")
                for r in range(8):
                    eng = nc.sync if r % 2 == 0 else nc.scalar
                    eng.dma_start(
                        out=wT[16 * r:16 * r + 16, :].rearrange(
                            "q (w f) -> q w f", f=FH),
                        in_=bass.AP(scrs[h], 0, [[FH, 16], [16 * FH, 8], [1, FH]]))
                wrp = wp.tile([P, FH * 8], i16, tag=f"wrp{h}")
                nc.vector.tensor_copy(
                    out=wrp[:].rearrange("p (f w) -> p w f", w=8),
                    in_=wT[:].rearrange("p (w f) -> p w f", f=FH))
                return CO, T3, CF0, wrp

            # ---------- per-half phase 2: weights --------------------------
            def phase2(h, CO, T3, CF0):
                CF1 = wp.tile([P, 3 * FH], f32, tag=f"CF1{h}")
                nc.vector.tensor_scalar(out=CF1[:], in0=T3[:], scalar1=1.0,
                                        scalar2=0.0, op0=OP.add, op1=OP.max)
                nc.vector.tensor_scalar(out=CF1[:], in0=CF1[:], scalar1=127.0,
                                        scalar2=None, op0=OP.min)
                nc.vector.tensor_scalar(out=CF1[:, 2 * FH:3 * FH],
                                        in0=CF1[:, 2 * FH:3 * FH], scalar1=64.0,
                                        scalar2=None, op0=OP.min)
                FB0 = wp.tile([P, 3 * FH], f32, tag=f"FB0{h}")   # fx0|fy0|fz0
                nc.vector.tensor_tensor(out=FB0[:], in0=CF1[:], in1=CO[:],
                                        op=OP.subtract)
                FB1 = wp.tile([P, 3 * FH], f32, tag=f"FB1{h}")   # fx1|fy1|fz1
                nc.vector.tensor_tensor(out=FB1[:], in0=CO[:], in1=CF0[:],
                                        op=OP.subtract)
                DXY = wp.tile([P, 2 * FH], f32, tag=f"DXY{h}")   # dx|dy
                nc.vector.tensor_tensor(out=DXY[:], in0=CF1[:, 0:2 * FH],
                                        in1=CF0[:, 0:2 * FH], op=OP.subtract)

                fx0, fx1 = FB0[:, 0:FH], FB1[:, 0:FH]
                fy0, fy1 = FB0[:, FH:2 * FH], FB1[:, FH:2 * FH]
                fz0, fz1 = FB0[:, 2 * FH:3 * FH], FB1[:, 2 * FH:3 * FH]
                dx, dy = DXY[:, 0:FH], DXY[:, FH:2 * FH]

                # zw = fz0*[Zf0==32] + fz1*[Zf1==32]
                e0 = wp.tile([P, FH], f32, tag=f"e0{h}")
                nc.vector.tensor_scalar(out=e0[:], in0=CF0[:, 2 * FH:3 * FH],
                                        scalar1=32.0, scalar2=None,
                                        op0=OP.is_equal)
                nc.vector.tensor_tensor(out=e0[:], in0=e0[:], in1=fz0, op=OP.mult)
                e1 = wp.tile([P, FH], f32, tag=f"e1{h}")
                nc.vector.tensor_scalar(out=e1[:], in0=CF1[:, 2 * FH:3 * FH],
                                        scalar1=32.0, scalar2=None,
                                        op0=OP.is_equal)
                nc.vector.tensor_tensor(out=e1[:], in0=e1[:], in1=fz1, op=OP.mult)
                zw = wp.tile([P, FH], f32, tag=f"zw{h}")
                nc.vector.tensor_tensor(out=zw[:], in0=e0[:], in1=e1[:], op=OP.add)

                # rf0 = (fx0 + (1-dy)*fx1)*zw ; rf1 = dy*fx1*zw
                # cf0 = fy0 + (1-dx)*fy1     ; cf1 = dx*fy1
                rf1 = wp.tile([P, FH], f32, tag=f"rf1{h}")
                nc.vector.tensor_tensor(out=rf1[:], in0=dy, in1=fx1, op=OP.mult)
                rf0 = wp.tile([P, FH], f32, tag=f"rf0{h}")
                nc.vector.tensor_tensor(out=rf0[:], in0=fx0, in1=fx1, op=OP.add)
                nc.vector.tensor_tensor(out=rf0[:], in0=rf0[:], in1=rf1[:],
                                        op=OP.subtract)
                nc.vector.tensor_tensor(out=rf0[:], in0=rf0[:], in1=zw[:],
                                        op=OP.mult)
                nc.vector.tensor_tensor(out=rf1[:], in0=rf1[:], in1=zw[:],
                                        op=OP.mult)
                cf1 = wp.tile([P, FH], f32, tag=f"cf1{h}")
                nc.vector.tensor_tensor(out=cf1[:], in0=dx, in1=fy1, op=OP.mult)
                cf0 = wp.tile([P, FH], f32, tag=f"cf0{h}")
                nc.vector.tensor_tensor(out=cf0[:], in0=fy0, in1=fy1, op=OP.add)
                nc.vector.tensor_tensor(out=cf0[:], in0=cf0[:], in1=cf1[:],
                                        op=OP.subtract)
                wfull = wp.tile([P, FH * 4], f32, tag=f"wfull{h}")
                for s, (a_, b_) in enumerate(((rf0, cf0), (rf0, cf1),
                                              (rf1, cf0), (rf1, cf1))):
                    nc.vector.tensor_tensor(out=wfull[:, s::4], in0=a_[:],
                                            in1=b_[:], op=OP.mult)
                if debug_taps:
                    nc.sync.dma_start(out=dbg_w[:, h * FH * 4:(h + 1) * FH * 4],
                                      in_=wfull[:])
                return wfull

            # ---------- per-half phase 3: gather + weighted reduce ---------
            def phase3(h, wrp, wfull):
                s0 = h * SH
                for sl in range(SH):        # one gather per pixel slot
                    f0 = sl * kw
                    g = gp.tile([P, kw * 4 * C], f32, tag="g")
                    # the final slot is gathered in two sub-gathers into the
                    # same tile: the last sub-transfer is half the size, so
                    # the exposed completion tail after the last descriptor
                    # generation shrinks (one mult+reduce still).
                    if sl == SH - 1 and not dense and kw % 2 == 0:
                        hk = kw // 2
                        for q2 in range(2):
                            nc.gpsimd.dma_gather(
                                out_ap=g[:, q2 * hk * 4 * C:(q2 + 1) * hk * 4 * C
                                         ].rearrange("p (k e) -> p k e", e=4 * C),
                                in_ap=tab[:],
                                idxs_ap=wrp[:, (f0 + q2 * hk) * 8:
                                            (f0 + (q2 + 1) * hk) * 8],
                                num_idxs=hk * P,
                                num_idxs_reg=hk * P,
                                elem_size=4 * C,
                                single_packet=False,
                            )
                    else:
                        nc.gpsimd.dma_gather(
                            out_ap=g[:].rearrange("p (k e) -> p k e", e=4 * C),
                            in_ap=tab[:],
                            idxs_ap=wrp[:, f0 * 8:(f0 + kw) * 8],
                            num_idxs=kw * P,
                            num_idxs_reg=kw * P,
                            elem_size=4 * C,
                            single_packet=False,
                        )
                    fr = kw * 4
                    tmp = tp.tile([P, kw * 4 * C], f32, tag="tmp")
                    nc.vector.tensor_tensor(
                        out=tmp[:].rearrange("p (c f) -> p f c", f=fr),
                        in0=g[:].rearrange("p (f c) -> p f c", c=C),
                        in1=wfull[:, f0 * 4:(f0 + kw) * 4].broadcast_to(
                            [P, fr, C]),
                        op=OP.mult)
                    o = op_.tile([P, C], f32, tag="o")
                    nc.vector.tensor_reduce(
                        out=o[:], in_=tmp[:].rearrange("p (c f) -> p c f", f=fr),
                        axis=mybir.AxisListType.X, op=OP.add)
                    nc.sync.dma_start(out=out_d[s0 + sl], in_=o[:])

            # ---------- emission order drives the pipeline -----------------
            rs = [phase1(h) for h in range(NH)]
            for h in range(NH):
                wf = phase2(h, rs[h][0], rs[h][1], rs[h][2])
                phase3(h, rs[h][3], wf)

    nc.compile()
    return nc


def _pick_kw(transformation):
    T = np.asarray(transformation, dtype=np.float32).reshape(3, 4)
    czk = abs(float(T[2, 2])) * 65.0 / 64.0   # |dz_voxel/dk|
    if czk == 0.0:
        return KD
    width = 2.0 / czk
    for kw in (6, 8, 12, 16, 24, 32, 48):
        if width <= kw - 1.5:
            return kw
    return KD


def _host_prep(image, transformation, kw):
    img = np.ascontiguousarray(np.asarray(image, dtype=np.float32)[0])  # (H, W, C)
    T = np.asarray(transformation, dtype=np.float32).reshape(3, 4)

    xp1 = np.minimum(np.arange(W) + 1, W - 1)
    yp1 = np.minimum(np.arange(H) + 1, H - 1)
    tab = np.concatenate(
        [img, img[:, xp1], img[yp1], img[yp1][:, xp1]], axis=2
    ).reshape(H * W, 4 * C)

    f = np.float32
    sp = [_split12(T[r, 2]) for r in range(3)]
    cols = [sp[0][0], sp[1][0], sp[2][0],          # 0-2: t2h xyz
            sp[0][1], sp[1][1], sp[2][1],          # 3-5: t2l xyz
            T[0, 3], T[1, 3], T[2, 3],             # 6-8: t3 xyz
            T[2, 2]]                               # 9: t2_z
    czk = f(T[2, 2] * f(32.5) / f(32.0))
    cols.append(f(1.0) / czk if czk != 0 else f(0.0))  # 10: rcz
    cols += [f(64.0), f(64.0), f(32.5)]            # 11-13: coord scales
    trep = np.tile(np.array(cols, dtype=f)[None, :], (P, 1))

    SH = NS // NH
    if kw == KD:
        jr = np.tile(((np.arange(KD) - 32) / 32).astype(f), (P, SH))
    else:
        jr = np.tile(np.arange(kw, dtype=f), (P, SH))

    in_maps = []
    for c in range(N_CORES):
        pix = c * 512 + np.arange(NS)[None, :] * P + np.arange(P)[:, None]  # (P, NS)
        xgp = XY_LIN[pix % OUT_W]
        ygp = XY_LIN[pix // OUT_W]
        b2 = np.empty((P, 3 * NS), dtype=f)
        for r in range(3):
            a1 = f(T[r, 0] * xgp)
            b2[:, r * NS:(r + 1) * NS] = _fma32(T[r, 1], ygp, a1)
        in_maps.append({
            "tab": tab,
            "trep": trep,
            "base2": b2,
            "jr": jr,
        })
    return in_maps


def _run(in_maps, kw, trace=False):
    nc = _CACHE.get(kw)
    if nc is None:
        nc = _build_program(kw)
        _CACHE[kw] = nc
    res = bass_utils.run_bass_kernel_spmd(
        nc, in_maps, core_ids=list(range(N_CORES)), trace=trace)
    out_full = np.empty((N_CORES * 512, C), dtype=np.float32)
    for c in range(N_CORES):
        out_full[c * 512:(c + 1) * 512] = res.results[c]["out"].reshape(512, C)
    return out_full.reshape(1, OUT_H, OUT_W, C), res


def kernel(image, transformation):
    kw = _pick_kw(transformation)
    in_maps = _host_prep(image, transformation, kw)
    out, _ = _run(in_maps, kw, trace=False)
    return out
